# revision 40
# baseline (speedup 1.0000x reference)
"""GCN classifier kernel for Trainium2, data-parallel over 8 NeuronCores.

Reference computation (per batch b):
    h = emb_table[sentences[b]]                      # [S, E]
    deg = adj[b].sum(-1) + 1                         # [S]
    for (W, bias):
        z = (adj[b] @ h + h) @ W.T + 2*bias          # [S, H]
        h = relu(z / deg[:, None])
    logits[b] = max_s(h) @ Wp.T + bp                 # [C]

On-device layout: h is kept transposed (hT: [feat, seq]).  With
A = adj + I (self-loop folded into the adjacency, done on host where
adj is also pre-transposed and cast to bf16):
    uT = matmul(lhsT=hT, rhs=WT)         # uT[t,j] = (h @ W.T)[t,j]
    zT = uT.T @ A.T                      # = (W @ hT) @ A.T = ((A h) W.T).T
so one accumulation group of 4 matmuls per layer does both the message
passing and the self-loop term.  The 2*bias is the per-partition bias of
the relu activation.  deg+1 falls out of ones.T @ A.T matmuls as a
[1, S] row, broadcast to all partitions by a ones x deg outer-product
matmul, inverted by one [128, S] DVE reciprocal.
Everything computes in bf16 with fp32 PSUM accumulation.
"""

import sys

import numpy as np

for _p in ("/opt/trn_rl_repo",):
    if _p not in sys.path:
        sys.path.insert(0, _p)

from contextlib import ExitStack

import ml_dtypes
import concourse.bass as bass
import concourse.mybir as mybir
import concourse.tile as tile
from concourse import bacc
from concourse._compat import with_exitstack
from concourse.bass_utils import run_bass_kernel_spmd
from concourse.masks import make_identity

B, S, E, H, V, C = 64, 512, 256, 128, 50000, 2
NCORES = 8
BL = B // NCORES  # batches per core

F32 = mybir.dt.float32
BF16 = mybir.dt.bfloat16
I32 = mybir.dt.int32

P = 128
S_TILES = S // P   # 4
E_TILES = E // P   # 2

BF16NP = ml_dtypes.bfloat16


@with_exitstack
def _gcn_tile_kernel(ctx: ExitStack, tc: tile.TileContext, aps: dict):
    nc = tc.nc
    sent = aps["sentences"]
    adjt = aps["adjt"]
    emb = aps["emb"]
    out = aps["out"]

    consts = ctx.enter_context(tc.tile_pool(name="consts", bufs=1))

    ident = consts.tile([P, P], BF16)
    make_identity(nc, ident[:])

    ones_col = consts.tile([P, 1], BF16)
    nc.gpsimd.memset(ones_col[:], 1.0)
    ones_row = consts.tile([1, P], BF16)
    nc.gpsimd.memset(ones_row[:], 1.0)
    ones8 = consts.tile([1, BL], BF16)
    nc.gpsimd.memset(ones8[:], 1.0)
    ones1 = consts.tile([1, 1], BF16)
    nc.gpsimd.memset(ones1[:], 1.0)
    zeros_bf = consts.tile([P, S], BF16)
    nc.gpsimd.memset(zeros_bf[:], 0.0)

    # consts go on the scalar HWDGE queue so the sync queue starts on
    # batch-0 adjT immediately
    w1t = consts.tile([P, E_TILES * P], BF16)  # W1.T as 2 k-tiles [128d, 128j]
    nc.scalar.dma_start(out=w1t[:].rearrange("p (k j) -> p k j", k=E_TILES),
                        in_=aps["w1t"].rearrange("(k p) j -> p k j", p=P))
    w2t = consts.tile([P, P], BF16)
    nc.scalar.dma_start(out=w2t[:], in_=aps["w2t"][:])
    w3t = consts.tile([P, P], BF16)
    nc.scalar.dma_start(out=w3t[:], in_=aps["w3t"][:])
    wpt = consts.tile([P, C], BF16)
    nc.scalar.dma_start(out=wpt[:], in_=aps["wpt"][:])
    bias_col = consts.tile([P, 3], F32)  # columns: 2*b1, 2*b2, 2*b3
    nc.scalar.dma_start(out=bias_col[:], in_=aps["bias2"][:])
    bpr = consts.tile([1, C], BF16)
    nc.scalar.dma_start(out=bpr[:], in_=aps["bpr"][:])

    pooledT = consts.tile([P, BL], BF16)  # max-pooled features, one col/batch

    adjT_p = ctx.enter_context(tc.tile_pool(name="adjT", bufs=4))
    h0_p = ctx.enter_context(tc.tile_pool(name="h0", bufs=4))
    hT_p = ctx.enter_context(tc.tile_pool(name="hT", bufs=3))
    uT_p = ctx.enter_context(tc.tile_pool(name="uT", bufs=3))
    tmp_p = ctx.enter_context(tc.tile_pool(name="tmp", bufs=3))
    idx_p = ctx.enter_context(tc.tile_pool(name="idx", bufs=8))
    r_p = ctx.enter_context(tc.tile_pool(name="r", bufs=3))

    ps_tr = ctx.enter_context(tc.tile_pool(name="ps_tr", bufs=2, space="PSUM"))
    ps_u = ctx.enter_context(tc.tile_pool(name="ps_u", bufs=2, space="PSUM"))
    ps_z = ctx.enter_context(tc.tile_pool(name="ps_z", bufs=3, space="PSUM"))
    ps_deg = ctx.enter_context(tc.tile_pool(name="ps_deg", bufs=1, space="PSUM"))

    def layer(hT_tiles, w_tiles, bias_ap, adjT_t, u_scale):
        """One GCN layer on unscaled hT tiles.  The 1/deg scale of the
        PREVIOUS layer's output rides the uT PSUM->SBUF copy (u_scale is
        r_col [128, S_TILES], r[t] at partition t%128, block t//128) —
        the relu epilogue is then a single ACT op, keeping the PE queue
        moving.  Returns hT_next = relu(zT + 2b), unscaled."""
        kt = len(hT_tiles)
        # uT[t, j] = sum_d hT[d, t] * WT[d, j]  (4 t-blocks side by side)
        psu = ps_u.tile([P, S], F32, tag="ps_u")
        for tt in range(S_TILES):
            for k in range(kt):
                nc.tensor.matmul(
                    out=psu[:, tt * P:(tt + 1) * P],
                    lhsT=hT_tiles[k][:, tt * P:(tt + 1) * P],
                    rhs=w_tiles[k][:],
                    start=(k == 0), stop=(k == kt - 1),
                )
        uT = uT_p.tile([P, S], BF16, tag="uT")
        if u_scale is None:
            nc.vector.tensor_copy(uT[:], psu[:])
        else:
            nc.vector.tensor_tensor(
                out=uT[:].rearrange("p (g j) -> p g j", g=S_TILES),
                in0=psu[:].rearrange("p (g j) -> p g j", g=S_TILES),
                in1=u_scale[:, :, None].broadcast_to([P, S_TILES, P]),
                op=mybir.AluOpType.mult,
            )

        # zT[j, s] = sum_t uT[t, j] A.T[t, s]   (A.T includes the +I term)
        psz = ps_z.tile([P, S], F32, tag="ps_z")
        for tt in range(S_TILES):
            nc.tensor.matmul(
                out=psz[:],
                lhsT=uT[:, tt * P:(tt + 1) * P],
                rhs=adjT_t[:, tt * S:(tt + 1) * S],
                start=(tt == 0), stop=(tt == S_TILES - 1),
            )

        # hT_next = relu(zT + 2b)   (the /deg scale is deferred).
        # Split halves across ACT and DVE so the chain hop is ~halved.
        hT_next = hT_p.tile([P, S], BF16, tag="hT")
        HS = S // 2
        nc.scalar.activation(hT_next[:, :HS], psz[:, :HS],
                             mybir.ActivationFunctionType.Relu, bias=bias_ap)
        nc.vector.scalar_tensor_tensor(
            out=hT_next[:, HS:], in0=psz[:, HS:], scalar=bias_ap,
            in1=zeros_bf[:, HS:], op0=mybir.AluOpType.add,
            op1=mybir.AluOpType.max,
        )
        return hT_next

    def issue_loads(b):
        """DMA-only staging: embedding gather + adjT load (no PE work).
        idx first so the serial gpsimd gather chain starts immediately."""
        idx = idx_p.tile([P, S_TILES], I32, tag="idx")
        for g in range(S_TILES):
            nc.sync.dma_start(out=idx[:, g:g + 1],
                              in_=sent[b, g * P:(g + 1) * P, None])
        adjT = adjT_p.tile([P, S_TILES * S], BF16, tag="adjT")
        nc.sync.dma_start(
            out=adjT[:].rearrange("p (g s) -> p g s", g=S_TILES),
            in_=adjt[b].rearrange("(g p) s -> p g s", p=P),
        )
        h0 = h0_p.tile([P, S_TILES * E], BF16, tag="h0")
        for g in range(S_TILES):
            nc.gpsimd.indirect_dma_start(
                out=h0[:, g * E:(g + 1) * E],
                out_offset=None,
                in_=emb[:],
                in_offset=bass.IndirectOffsetOnAxis(ap=idx[:, g:g + 1], axis=0),
            )
        return adjT, h0

    def issue_deg(adjT):
        """deg[s] = sum_t A.T[t, s] -> 1/deg in two layouts: r_col
        [128, S_TILES] (partition-major, scales uT rows) and r_bc
        [128, S] (free-major broadcast, scales the last layer's output
        before pooling)."""
        psd = ps_deg.tile([1, S], F32, tag="ps_deg")
        for g in range(S_TILES):
            nc.tensor.matmul(
                out=psd[:], lhsT=ones_col[:], rhs=adjT[:, g * S:(g + 1) * S],
                start=(g == 0), stop=(g == S_TILES - 1),
            )
        deg_bf = r_p.tile([1, S], BF16, tag="deg")
        nc.scalar.copy(deg_bf[:], psd[:])
        # partition-major: deg column blocks via K=1 matmuls with ones[1,1]
        # (shares the ps_deg bank slot; the chain is serial anyway)
        ps_dc = ps_deg.tile([P, S_TILES], F32, tag="ps_deg")
        for g in range(S_TILES):
            nc.tensor.matmul(out=ps_dc[:, g:g + 1],
                             lhsT=deg_bf[:, g * P:(g + 1) * P], rhs=ones1[:],
                             start=True, stop=True)
        r_col = r_p.tile([P, S_TILES], F32, tag="rcol")
        nc.vector.reciprocal_approx_fast(out=r_col[:], in_=ps_dc[:])
        # free-major broadcast (ones x deg), for the pre-pool scale
        ps_rb = ps_u.tile([P, S], F32, tag="ps_u")
        nc.tensor.matmul(out=ps_rb[:], lhsT=ones_row[:], rhs=deg_bf[:],
                         start=True, stop=True)
        r_bc = r_p.tile([P, S], F32, tag="rbc")
        nc.vector.reciprocal_approx_fast(out=r_bc[:], in_=ps_rb[:])
        return r_col, r_bc

    def issue_h0T(h0):
        """Transpose gathered embeddings to [feat, seq] (PE + DVE)."""
        h0T = hT_p.tile([P, E_TILES * S], BF16, tag="h0T")
        for dd in range(E_TILES):
            pst = ps_tr.tile([P, S], BF16, tag="ps_tr")
            for g in range(S_TILES):
                nc.tensor.transpose(
                    out=pst[:, g * P:(g + 1) * P],
                    in_=h0[:, g * E + dd * P: g * E + (dd + 1) * P],
                    identity=ident[:],
                )
            nc.vector.tensor_copy(h0T[:, dd * S:(dd + 1) * S], pst[:])
        return h0T

    # PE warm-up burst: ~4us of dense back-to-back matmuls on scratch data
    # while the first DMAs are in flight, so the HAM clock promotes to 2.4GHz
    # before batch 0's real math starts.
    warm_s = consts.tile([P, S], BF16)
    nc.gpsimd.memset(warm_s[:], 0.0)
    ps_w = ps_deg.tile([1, S], F32, tag="ps_deg")
    N_WARM = 14
    for i in range(N_WARM):
        nc.tensor.matmul(out=ps_w[:], lhsT=ones_col[:], rhs=warm_s[:],
                         start=(i == 0), stop=(i == N_WARM - 1))

    # Software pipeline with the next batch's PE work (h0 transposes, deg)
    # emitted BETWEEN this batch's layers: the in-order PE queue then always
    # has ready work at each layer-epilogue boundary, which also keeps the HAM
    # clock at full rate.
    loads = issue_loads(0)
    adjT0, h00 = loads
    r0 = issue_deg(adjT0)
    cur = (adjT0, issue_h0T(h00), r0)
    nxt_loads = issue_loads(1)
    nxt = {}
    for b in range(BL):
        adjT, h0T, (r_col, r_bc) = cur

        h1 = layer([h0T[:, :S], h0T[:, S:]],
                   [w1t[:, :P], w1t[:, P:]],
                   bias_col[:, 0:1], adjT[:], None)
        if nxt_loads is not None:
            nxt["h0T"] = issue_h0T(nxt_loads[1])

        h2 = layer([h1[:]], [w2t[:]], bias_col[:, 1:2], adjT[:], r_col[:])
        if nxt_loads is not None:
            nxt["r"] = issue_deg(nxt_loads[0])
        if b + 2 < BL:
            nxt2_loads = issue_loads(b + 2)
        else:
            nxt2_loads = None

        h3 = layer([h2[:]], [w3t[:]], bias_col[:, 2:3], adjT[:], r_col[:])

        # apply the deferred 1/deg to the last layer's output, then pool
        h3s = tmp_p.tile([P, S], BF16, tag="tmp")
        nc.vector.tensor_tensor(out=h3s[:], in0=h3[:], in1=r_bc[:],
                                op=mybir.AluOpType.mult)
        nc.vector.reduce_max(pooledT[:, b:b + 1], h3s[:],
                             axis=mybir.AxisListType.X)
        if nxt_loads is not None:
            cur = (nxt_loads[0], nxt["h0T"], nxt["r"])
        nxt_loads = nxt2_loads

    # ---- classifier: logits = pooled @ Wp.T + bp ----
    psl = ps_z.tile([BL, C], F32, tag="ps_z")
    nc.tensor.matmul(out=psl[:], lhsT=pooledT[:], rhs=wpt[:],
                     start=True, stop=False)
    nc.tensor.matmul(out=psl[:], lhsT=ones8[:], rhs=bpr[:],
                     start=False, stop=True)
    logits = consts.tile([BL, C], F32)
    nc.vector.tensor_copy(logits[:], psl[:])
    nc.sync.dma_start(out=out[:], in_=logits[:])


def build_bass():
    nc = bacc.Bacc("TRN2", target_bir_lowering=False, debug=False)
    aps = {
        "sentences": nc.dram_tensor("sentences", [BL, S], I32,
                                    kind="ExternalInput").ap(),
        "adjt": nc.dram_tensor("adjt", [BL, S, S], BF16,
                               kind="ExternalInput").ap(),
        "emb": nc.dram_tensor("emb", [V, E], BF16, kind="ExternalInput").ap(),
        "w1t": nc.dram_tensor("w1t", [E, H], BF16, kind="ExternalInput").ap(),
        "w2t": nc.dram_tensor("w2t", [H, H], BF16, kind="ExternalInput").ap(),
        "w3t": nc.dram_tensor("w3t", [H, H], BF16, kind="ExternalInput").ap(),
        "wpt": nc.dram_tensor("wpt", [H, C], BF16, kind="ExternalInput").ap(),
        "bias2": nc.dram_tensor("bias2", [H, 3], F32,
                                kind="ExternalInput").ap(),
        "bpr": nc.dram_tensor("bpr", [1, C], BF16, kind="ExternalInput").ap(),
        "out": nc.dram_tensor("out", [BL, C], F32,
                              kind="ExternalOutput").ap(),
    }
    with tile.TileContext(nc) as tc:
        _gcn_tile_kernel(tc, aps)
    nc.compile()
    return nc


_NC_CACHE = None


def _get_nc():
    global _NC_CACHE
    if _NC_CACHE is None:
        _NC_CACHE = build_bass()
    return _NC_CACHE


def make_in_maps(sentences, adj, emb_table, W1, b1, W2, b2, W3, b3, Wp, bp):
    sentences = np.ascontiguousarray(np.asarray(sentences).astype(np.int32))
    # A.T = adj.T + I: fold the self-loop into the adjacency
    adjt = np.asarray(adj, dtype=np.float32).transpose(0, 2, 1).copy()
    _ar = np.arange(S)
    adjt[:, _ar, _ar] += 1.0
    adjt = np.ascontiguousarray(adjt.astype(BF16NP))
    emb_bf = np.ascontiguousarray(np.asarray(emb_table,
                                             dtype=np.float32).astype(BF16NP))
    w1t = np.ascontiguousarray(np.asarray(W1, dtype=np.float32).T.astype(BF16NP))
    w2t = np.ascontiguousarray(np.asarray(W2, dtype=np.float32).T.astype(BF16NP))
    w3t = np.ascontiguousarray(np.asarray(W3, dtype=np.float32).T.astype(BF16NP))
    wpt = np.ascontiguousarray(np.asarray(Wp, dtype=np.float32).T.astype(BF16NP))
    bias2 = np.ascontiguousarray(
        np.stack([2.0 * np.asarray(b1, dtype=np.float32),
                  2.0 * np.asarray(b2, dtype=np.float32),
                  2.0 * np.asarray(b3, dtype=np.float32)], axis=1))
    bpr = np.ascontiguousarray(
        np.asarray(bp, dtype=np.float32)[None, :].astype(BF16NP))

    in_maps = []
    for c in range(NCORES):
        sl = slice(c * BL, (c + 1) * BL)
        in_maps.append({
            "sentences": np.ascontiguousarray(sentences[sl]),
            "adjt": np.ascontiguousarray(adjt[sl]),
            "emb": emb_bf,
            "w1t": w1t, "w2t": w2t, "w3t": w3t, "wpt": wpt,
            "bias2": bias2, "bpr": bpr,
        })
    return in_maps


def run(in_maps, trace=False, **kw):
    nc = _get_nc()
    return run_bass_kernel_spmd(nc, in_maps, list(range(NCORES)),
                                trace=trace, **kw)


def kernel(sentences, adj, emb_table, W1, b1, W2, b2, W3, b3, Wp, bp):
    in_maps = make_in_maps(sentences, adj, emb_table,
                           W1, b1, W2, b2, W3, b3, Wp, bp)
    res = run(in_maps)
    return np.concatenate([res.results[c]["out"] for c in range(NCORES)],
                          axis=0)


# revision 41
# speedup vs baseline: 1.1485x; 1.1485x over previous
"""GCN classifier kernel for Trainium2, data-parallel over 8 NeuronCores.

Reference computation (per batch b):
    h = emb_table[sentences[b]]                      # [S, E]
    deg = adj[b].sum(-1) + 1                         # [S]
    for (W, bias):
        z = (adj[b] @ h + h) @ W.T + 2*bias          # [S, H]
        h = relu(z / deg[:, None])
    logits[b] = max_s(h) @ Wp.T + bp                 # [C]

On-device layout: h is kept transposed (hT: [feat, seq]).  With
A = adj + I (self-loop folded into the adjacency, done on host where
adj is also pre-transposed and cast to bf16):
    uT = matmul(lhsT=hT, rhs=WT)         # uT[t,j] = (h @ W.T)[t,j]
    zT = uT.T @ A.T                      # = (W @ hT) @ A.T = ((A h) W.T).T
so one accumulation group of 4 matmuls per layer does both the message
passing and the self-loop term.  The 2*bias is the per-partition bias of
the relu activation.  deg+1 falls out of ones.T @ A.T matmuls as a
[1, S] row, broadcast to all partitions by a ones x deg outer-product
matmul, inverted by one [128, S] DVE reciprocal.
Everything computes in bf16 with fp32 PSUM accumulation.
"""

import sys

import numpy as np

for _p in ("/opt/trn_rl_repo",):
    if _p not in sys.path:
        sys.path.insert(0, _p)

from contextlib import ExitStack

import ml_dtypes
import concourse.bass as bass
import concourse.mybir as mybir
import concourse.tile as tile
from concourse import bacc
from concourse._compat import with_exitstack
from concourse.bass_utils import run_bass_kernel_spmd
from concourse.masks import make_identity

B, S, E, H, V, C = 64, 512, 256, 128, 50000, 2
NCORES = 8
BL = B // NCORES  # batches per core

F32 = mybir.dt.float32
BF16 = mybir.dt.bfloat16
I32 = mybir.dt.int32

P = 128
S_TILES = S // P   # 4
E_TILES = E // P   # 2

BF16NP = ml_dtypes.bfloat16


@with_exitstack
def _gcn_tile_kernel(ctx: ExitStack, tc: tile.TileContext, aps: dict):
    nc = tc.nc
    sent = aps["sentences"]
    adjt = aps["adjt"]
    emb = aps["emb"]
    out = aps["out"]

    consts = ctx.enter_context(tc.tile_pool(name="consts", bufs=1))

    ident = consts.tile([P, P], BF16)
    make_identity(nc, ident[:])

    ones_col = consts.tile([P, 1], BF16)
    nc.gpsimd.memset(ones_col[:], 1.0)
    ones_row = consts.tile([1, P], BF16)
    nc.gpsimd.memset(ones_row[:], 1.0)
    ones8 = consts.tile([1, BL], BF16)
    nc.gpsimd.memset(ones8[:], 1.0)
    ones1 = consts.tile([1, 1], BF16)
    nc.gpsimd.memset(ones1[:], 1.0)
    zeros_bf = consts.tile([P, S], BF16)
    nc.gpsimd.memset(zeros_bf[:], 0.0)

    # consts go on the scalar HWDGE queue so the sync queue starts on
    # batch-0 adjT immediately
    w1t = consts.tile([P, E_TILES * P], BF16)  # W1.T as 2 k-tiles [128d, 128j]
    nc.scalar.dma_start(out=w1t[:].rearrange("p (k j) -> p k j", k=E_TILES),
                        in_=aps["w1t"].rearrange("(k p) j -> p k j", p=P))
    w2t = consts.tile([P, P], BF16)
    nc.scalar.dma_start(out=w2t[:], in_=aps["w2t"][:])
    w3t = consts.tile([P, P], BF16)
    nc.scalar.dma_start(out=w3t[:], in_=aps["w3t"][:])
    wpt = consts.tile([P, C], BF16)
    nc.scalar.dma_start(out=wpt[:], in_=aps["wpt"][:])
    bias_col = consts.tile([P, 3], F32)  # columns: 2*b1, 2*b2, 2*b3
    nc.scalar.dma_start(out=bias_col[:], in_=aps["bias2"][:])
    bpr = consts.tile([1, C], BF16)
    nc.scalar.dma_start(out=bpr[:], in_=aps["bpr"][:])

    pooledT = consts.tile([P, BL], BF16)  # max-pooled features, one col/batch

    adjT_p = ctx.enter_context(tc.tile_pool(name="adjT", bufs=4))
    h0_p = ctx.enter_context(tc.tile_pool(name="h0", bufs=4))
    hT_p = ctx.enter_context(tc.tile_pool(name="hT", bufs=3))
    uT_p = ctx.enter_context(tc.tile_pool(name="uT", bufs=3))
    tmp_p = ctx.enter_context(tc.tile_pool(name="tmp", bufs=3))
    idx_p = ctx.enter_context(tc.tile_pool(name="idx", bufs=8))
    r_p = ctx.enter_context(tc.tile_pool(name="r", bufs=3))

    ps_tr = ctx.enter_context(tc.tile_pool(name="ps_tr", bufs=2, space="PSUM"))
    ps_u = ctx.enter_context(tc.tile_pool(name="ps_u", bufs=2, space="PSUM"))
    ps_z = ctx.enter_context(tc.tile_pool(name="ps_z", bufs=3, space="PSUM"))
    ps_deg = ctx.enter_context(tc.tile_pool(name="ps_deg", bufs=1, space="PSUM"))

    def layer(hT_tiles, w_tiles, bias_ap, adjT_t, u_scale):
        """One GCN layer on unscaled hT tiles.  The 1/deg scale of the
        PREVIOUS layer's output rides the uT PSUM->SBUF copy (u_scale is
        r_col [128, S_TILES], r[t] at partition t%128, block t//128) —
        the relu epilogue is then a single ACT op, keeping the PE queue
        moving.  Returns hT_next = relu(zT + 2b), unscaled."""
        kt = len(hT_tiles)
        # uT[t, j] = sum_d hT[d, t] * WT[d, j]  (4 t-blocks side by side)
        psu = ps_u.tile([P, S], F32, tag="ps_u")
        for tt in range(S_TILES):
            for k in range(kt):
                nc.tensor.matmul(
                    out=psu[:, tt * P:(tt + 1) * P],
                    lhsT=hT_tiles[k][:, tt * P:(tt + 1) * P],
                    rhs=w_tiles[k][:],
                    start=(k == 0), stop=(k == kt - 1),
                )
        uT = uT_p.tile([P, S], BF16, tag="uT")
        if u_scale is None:
            nc.vector.tensor_copy(uT[:], psu[:])
        else:
            nc.vector.tensor_tensor(
                out=uT[:].rearrange("p (g j) -> p g j", g=S_TILES),
                in0=psu[:].rearrange("p (g j) -> p g j", g=S_TILES),
                in1=u_scale[:, :, None].broadcast_to([P, S_TILES, P]),
                op=mybir.AluOpType.mult,
            )

        # zT[j, s] = sum_t uT[t, j] A.T[t, s]   (A.T includes the +I term)
        psz = ps_z.tile([P, S], F32, tag="ps_z")
        for tt in range(S_TILES):
            nc.tensor.matmul(
                out=psz[:],
                lhsT=uT[:, tt * P:(tt + 1) * P],
                rhs=adjT_t[:, tt * S:(tt + 1) * S],
                start=(tt == 0), stop=(tt == S_TILES - 1),
            )

        # hT_next = relu(zT + 2b)   (the /deg scale is deferred)
        hT_next = hT_p.tile([P, S], BF16, tag="hT")
        nc.scalar.activation(hT_next[:], psz[:],
                             mybir.ActivationFunctionType.Relu, bias=bias_ap)
        return hT_next

    def issue_loads(b):
        """DMA-only staging: embedding gather + adjT load (no PE work).
        idx first so the serial gpsimd gather chain starts immediately."""
        idx = idx_p.tile([P, S_TILES], I32, tag="idx")
        for g in range(S_TILES):
            nc.sync.dma_start(out=idx[:, g:g + 1],
                              in_=sent[b, g * P:(g + 1) * P, None])
        adjT = adjT_p.tile([P, S_TILES * S], BF16, tag="adjT")
        nc.sync.dma_start(
            out=adjT[:].rearrange("p (g s) -> p g s", g=S_TILES),
            in_=adjt[b].rearrange("(g p) s -> p g s", p=P),
        )
        h0 = h0_p.tile([P, S_TILES * E], BF16, tag="h0")
        for g in range(S_TILES):
            nc.gpsimd.indirect_dma_start(
                out=h0[:, g * E:(g + 1) * E],
                out_offset=None,
                in_=emb[:],
                in_offset=bass.IndirectOffsetOnAxis(ap=idx[:, g:g + 1], axis=0),
            )
        return adjT, h0

    def issue_deg(adjT):
        """deg[s] = sum_t A.T[t, s] -> 1/deg in two layouts: r_col
        [128, S_TILES] (partition-major, scales uT rows) and r_bc
        [128, S] (free-major broadcast, scales the last layer's output
        before pooling)."""
        psd = ps_deg.tile([1, S], F32, tag="ps_deg")
        for g in range(S_TILES):
            nc.tensor.matmul(
                out=psd[:], lhsT=ones_col[:], rhs=adjT[:, g * S:(g + 1) * S],
                start=(g == 0), stop=(g == S_TILES - 1),
            )
        deg_bf = r_p.tile([1, S], BF16, tag="deg")
        nc.scalar.copy(deg_bf[:], psd[:])
        # partition-major: deg column blocks via K=1 matmuls with ones[1,1]
        # (shares the ps_deg bank slot; the chain is serial anyway)
        ps_dc = ps_deg.tile([P, S_TILES], F32, tag="ps_deg")
        for g in range(S_TILES):
            nc.tensor.matmul(out=ps_dc[:, g:g + 1],
                             lhsT=deg_bf[:, g * P:(g + 1) * P], rhs=ones1[:],
                             start=True, stop=True)
        r_col = r_p.tile([P, S_TILES], F32, tag="rcol")
        nc.vector.reciprocal_approx_fast(out=r_col[:], in_=ps_dc[:])
        # free-major broadcast (ones x deg), for the pre-pool scale
        ps_rb = ps_u.tile([P, S], F32, tag="ps_u")
        nc.tensor.matmul(out=ps_rb[:], lhsT=ones_row[:], rhs=deg_bf[:],
                         start=True, stop=True)
        r_bc = r_p.tile([P, S], F32, tag="rbc")
        nc.vector.reciprocal_approx_fast(out=r_bc[:], in_=ps_rb[:])
        return r_col, r_bc

    def issue_h0T(h0):
        """Transpose gathered embeddings to [feat, seq] (PE + DVE)."""
        h0T = hT_p.tile([P, E_TILES * S], BF16, tag="h0T")
        for dd in range(E_TILES):
            pst = ps_tr.tile([P, S], BF16, tag="ps_tr")
            for g in range(S_TILES):
                nc.tensor.transpose(
                    out=pst[:, g * P:(g + 1) * P],
                    in_=h0[:, g * E + dd * P: g * E + (dd + 1) * P],
                    identity=ident[:],
                )
            nc.vector.tensor_copy(h0T[:, dd * S:(dd + 1) * S], pst[:])
        return h0T

    # PE warm-up burst: ~4us of dense back-to-back matmuls on scratch data
    # while the first DMAs are in flight, so the HAM clock promotes to 2.4GHz
    # before batch 0's real math starts.
    warm_s = consts.tile([P, S], BF16)
    nc.gpsimd.memset(warm_s[:], 0.0)
    ps_w = ps_deg.tile([1, S], F32, tag="ps_deg")
    N_WARM = 14
    for i in range(N_WARM):
        nc.tensor.matmul(out=ps_w[:], lhsT=ones_col[:], rhs=warm_s[:],
                         start=(i == 0), stop=(i == N_WARM - 1))

    # Software pipeline with the next batch's PE work (h0 transposes, deg)
    # emitted BETWEEN this batch's layers: the in-order PE queue then always
    # has ready work at each layer-epilogue boundary, which also keeps the HAM
    # clock at full rate.
    loads = issue_loads(0)
    adjT0, h00 = loads
    r0 = issue_deg(adjT0)
    cur = (adjT0, issue_h0T(h00), r0)
    nxt_loads = issue_loads(1)
    nxt = {}
    for b in range(BL):
        adjT, h0T, (r_col, r_bc) = cur

        h1 = layer([h0T[:, :S], h0T[:, S:]],
                   [w1t[:, :P], w1t[:, P:]],
                   bias_col[:, 0:1], adjT[:], None)
        if nxt_loads is not None:
            nxt["h0T"] = issue_h0T(nxt_loads[1])

        h2 = layer([h1[:]], [w2t[:]], bias_col[:, 1:2], adjT[:], r_col[:])
        if nxt_loads is not None:
            nxt["r"] = issue_deg(nxt_loads[0])
        if b + 2 < BL:
            nxt2_loads = issue_loads(b + 2)
        else:
            nxt2_loads = None

        h3 = layer([h2[:]], [w3t[:]], bias_col[:, 2:3], adjT[:], r_col[:])

        # apply the deferred 1/deg to the last layer's output, then pool
        h3s = tmp_p.tile([P, S], BF16, tag="tmp")
        nc.vector.tensor_tensor(out=h3s[:], in0=h3[:], in1=r_bc[:],
                                op=mybir.AluOpType.mult)
        nc.vector.reduce_max(pooledT[:, b:b + 1], h3s[:],
                             axis=mybir.AxisListType.X)
        if nxt_loads is not None:
            cur = (nxt_loads[0], nxt["h0T"], nxt["r"])
        nxt_loads = nxt2_loads

    # ---- classifier: logits = pooled @ Wp.T + bp ----
    psl = ps_z.tile([BL, C], F32, tag="ps_z")
    nc.tensor.matmul(out=psl[:], lhsT=pooledT[:], rhs=wpt[:],
                     start=True, stop=False)
    nc.tensor.matmul(out=psl[:], lhsT=ones8[:], rhs=bpr[:],
                     start=False, stop=True)
    logits = consts.tile([BL, C], F32)
    nc.vector.tensor_copy(logits[:], psl[:])
    nc.sync.dma_start(out=out[:], in_=logits[:])


def build_bass():
    nc = bacc.Bacc("TRN2", target_bir_lowering=False, debug=False)
    aps = {
        "sentences": nc.dram_tensor("sentences", [BL, S], I32,
                                    kind="ExternalInput").ap(),
        "adjt": nc.dram_tensor("adjt", [BL, S, S], BF16,
                               kind="ExternalInput").ap(),
        "emb": nc.dram_tensor("emb", [V, E], BF16, kind="ExternalInput").ap(),
        "w1t": nc.dram_tensor("w1t", [E, H], BF16, kind="ExternalInput").ap(),
        "w2t": nc.dram_tensor("w2t", [H, H], BF16, kind="ExternalInput").ap(),
        "w3t": nc.dram_tensor("w3t", [H, H], BF16, kind="ExternalInput").ap(),
        "wpt": nc.dram_tensor("wpt", [H, C], BF16, kind="ExternalInput").ap(),
        "bias2": nc.dram_tensor("bias2", [H, 3], F32,
                                kind="ExternalInput").ap(),
        "bpr": nc.dram_tensor("bpr", [1, C], BF16, kind="ExternalInput").ap(),
        "out": nc.dram_tensor("out", [BL, C], F32,
                              kind="ExternalOutput").ap(),
    }
    with tile.TileContext(nc) as tc:
        _gcn_tile_kernel(tc, aps)
    nc.compile()
    return nc


_NC_CACHE = None


def _get_nc():
    global _NC_CACHE
    if _NC_CACHE is None:
        _NC_CACHE = build_bass()
    return _NC_CACHE


def make_in_maps(sentences, adj, emb_table, W1, b1, W2, b2, W3, b3, Wp, bp):
    sentences = np.ascontiguousarray(np.asarray(sentences).astype(np.int32))
    # A.T = adj.T + I: fold the self-loop into the adjacency
    adjt = np.asarray(adj, dtype=np.float32).transpose(0, 2, 1).copy()
    _ar = np.arange(S)
    adjt[:, _ar, _ar] += 1.0
    adjt = np.ascontiguousarray(adjt.astype(BF16NP))
    emb_bf = np.ascontiguousarray(np.asarray(emb_table,
                                             dtype=np.float32).astype(BF16NP))
    w1t = np.ascontiguousarray(np.asarray(W1, dtype=np.float32).T.astype(BF16NP))
    w2t = np.ascontiguousarray(np.asarray(W2, dtype=np.float32).T.astype(BF16NP))
    w3t = np.ascontiguousarray(np.asarray(W3, dtype=np.float32).T.astype(BF16NP))
    wpt = np.ascontiguousarray(np.asarray(Wp, dtype=np.float32).T.astype(BF16NP))
    bias2 = np.ascontiguousarray(
        np.stack([2.0 * np.asarray(b1, dtype=np.float32),
                  2.0 * np.asarray(b2, dtype=np.float32),
                  2.0 * np.asarray(b3, dtype=np.float32)], axis=1))
    bpr = np.ascontiguousarray(
        np.asarray(bp, dtype=np.float32)[None, :].astype(BF16NP))

    in_maps = []
    for c in range(NCORES):
        sl = slice(c * BL, (c + 1) * BL)
        in_maps.append({
            "sentences": np.ascontiguousarray(sentences[sl]),
            "adjt": np.ascontiguousarray(adjt[sl]),
            "emb": emb_bf,
            "w1t": w1t, "w2t": w2t, "w3t": w3t, "wpt": wpt,
            "bias2": bias2, "bpr": bpr,
        })
    return in_maps


def run(in_maps, trace=False, **kw):
    nc = _get_nc()
    return run_bass_kernel_spmd(nc, in_maps, list(range(NCORES)),
                                trace=trace, **kw)


def kernel(sentences, adj, emb_table, W1, b1, W2, b2, W3, b3, Wp, bp):
    in_maps = make_in_maps(sentences, adj, emb_table,
                           W1, b1, W2, b2, W3, b3, Wp, bp)
    res = run(in_maps)
    return np.concatenate([res.results[c]["out"] for c in range(NCORES)],
                          axis=0)
